# revision 15
# baseline (speedup 1.0000x reference)
"""Trainium2 Bass kernel for the "no two consecutive > threshold" recurrence.

Reference semantics (per row, scanning along the seq axis S):
    out[0] = x[0]
    out[t] = x[t] * (1 - (out[t-1] > 0.5) * (x[t] > 0.5))

Key transformation: with d0[t] = 0.5 + (x[t] <= 0.5)  (i.e. 1.5 for small x,
0.5 for large x), the recurrence is exactly

    out[t] = x[t] * (d0[t] >= out[t-1])

because out[t-1] < 1.0 always (so d0 = 1.5 always passes), and d0 = 0.5
implements the (out[t-1] > 0.5) kill test. This maps 1:1 onto the DVE
``tensor_tensor_scan`` instruction:

    state = (data0[:,t] op0 state) op1 data1[:,t]
          = (d0[:,t] is_ge state) mult x[:,t]

so the whole kernel is, per [128, S] tile: one fused tensor_scalar (DVE,
2x_2P mode) to build d0, one tensor_tensor_scan (DVE, half-throughput
stock op) that directly produces the final output, and the two DMAs.
Real-HW measured ~86us/core steady state (DVE-bound; DMA 64us hidden);
GPSIMD/ACT/PE cannot take any of this work (measured/compiler-verified).

Sharding: embarrassingly data-parallel over the batch axis -- 4096 rows
split as 8 x 512 contiguous row blocks, one per NeuronCore.
"""

import numpy as np

_B, _S = 4096, 8192  # full input shape [B, S] float32
_NC = 8  # NeuronCores
_RPC = _B // _NC  # rows per core = 512
_P = 128  # SBUF partitions
_NT = _RPC // _P  # row tiles per core = 4

_cache = {}

# Tunables (chosen via TimelineSim sweeps: chunks=2/bufs=4 hits the DMA-only
# floor of 96.6us; chunks=1 pays ~11us of pipeline fill/drain).
_CHUNKS = 2  # seq chunks per [128, S] row tile
_XBUFS = 4
_DBUFS = 4


def _build(chunks=_CHUNKS, xbufs=_XBUFS, dbufs=_DBUFS, repeat=1,
           variable_edges=True):
    import concourse.bacc as bacc
    import concourse.mybir as mybir
    from concourse.tile import TileContext

    Alu = mybir.AluOpType
    f32 = mybir.dt.float32
    cw = _S // chunks  # chunk width along seq

    nc = bacc.Bacc("TRN2", debug=False, num_devices=_NC)
    x_d = nc.dram_tensor("x", (_RPC, _S), f32, kind="ExternalInput").ap()
    y_d = nc.dram_tensor("y", (_RPC, _S), f32, kind="ExternalOutput").ap()

    # Per-row-tile seq chunk widths. The very first chunk (tile 0) and very
    # last chunk (tile NT-1) are small so the single-shot pipeline fill
    # (first load before DVE can start) and drain (last store) are short;
    # steady-state DVE work is unchanged.
    base = [cw] * chunks
    if variable_edges:
        widths = {0: [1024, cw - 1024] + [cw] * (chunks - 1),
                  _NT - 1: [cw] * (chunks - 1) + [cw - 1024, 1024]}
    else:
        widths = {}

    with TileContext(nc) as tc:
        with tc.tile_pool(name="sbuf", bufs=2) as pool:
            # [P,1] bias plane for the Sign activation (non-Copy activation
            # funcs need an AP bias, and no 0.5 const AP is registered).
            halfb = pool.tile([_P, 1], f32, tag="hb", bufs=1, name="halfb")
            nc.vector.memset(halfb[:], 0.5)
            for rep in range(repeat):
                for i in range(_NT):
                    r0, r1 = i * _P, (i + 1) * _P
                    prev = None  # previous chunk's output tile (for scan carry)
                    prev_w = 0
                    offs = 0
                    for c, w in enumerate(widths.get(i, base)):
                        s0, s1 = offs, offs + w
                        offs = s1
                        xt = pool.tile([_P, w], f32, tag="x", bufs=xbufs,
                                       name=f"xt{rep}_{i}_{c}")
                        nc.sync.dma_start(out=xt[:], in_=x_d[r0:r1, s0:s1])
                        # d0: 1.5/1.0 where x <= 0.5 (keep-always: states are
                        # strictly < 1.0), 0.5 where x > 0.5 (test prev>0.5).
                        # Built on the otherwise-idle ACT engine as two chained
                        # activations (Sign and Copy share every table set, so
                        # no table reloads):
                        #   s  = Sign(0.5 - x)   in {+1, 0 at x=0.5, -1}
                        #   d0 = 0.5*s + 1.0     in {1.5, 1.0, 0.5}
                        # This keeps DVE for scans only (~68us vs ~86us when
                        # DVE also ran the d0 tensor_scalar). Sign(0) must not
                        # be -1 on HW (0 or +1 both correct); verified exact
                        # against the reference including x == 0.5 elements.
                        d0 = pool.tile([_P, w], f32, tag="d", bufs=dbufs,
                                       name=f"d{rep}_{i}_{c}")
                        Act = mybir.ActivationFunctionType
                        nc.scalar.activation(
                            out=d0[:], in_=xt[:], func=Act.Sign,
                            bias=halfb[:], scale=-1.0,
                        )
                        nc.scalar.activation(
                            out=d0[:], in_=d0[:], func=Act.Copy,
                            bias=1.0, scale=0.5,
                        )
                        # out[t] = (d0[t] >= out[t-1]) * x[t]; in place over d0.
                        # Carry across chunks: initial = prev chunk's last col.
                        init = 0.0 if prev is None else prev[:, prev_w - 1:prev_w]
                        nc.vector.tensor_tensor_scan(
                            out=d0[:], data0=d0[:], data1=xt[:], initial=init,
                            op0=Alu.is_ge, op1=Alu.mult,
                        )
                        nc.sync.dma_start(out=y_d[r0:r1, s0:s1], in_=d0[:])
                        prev = d0
                        prev_w = w

    nc.compile()
    return nc


def _get_nc():
    if "nc" not in _cache:
        _cache["nc"] = _build()
    return _cache["nc"]


def _run(x, trace=False):
    from concourse.bass_utils import run_bass_kernel_spmd

    nc = _get_nc()
    x = np.ascontiguousarray(np.asarray(x, dtype=np.float32))
    assert x.shape == (_B, _S), x.shape
    in_maps = [
        {"x": np.ascontiguousarray(x[k * _RPC:(k + 1) * _RPC])} for k in range(_NC)
    ]
    res = run_bass_kernel_spmd(nc, in_maps, list(range(_NC)), trace=trace)
    out = np.concatenate([res.results[k]["y"] for k in range(_NC)], axis=0)
    return out, res


def kernel(x):
    out, _ = _run(x, trace=False)
    return out
